# revision 10
# baseline (speedup 1.0000x reference)
"""GroupNorm + single-head-per-core attention + output projection for
nn_Attention_55697135894780 on 8 TRN2 NeuronCores.

Sharding: one (batch, head) pair per core (B=2 x NH=4 = 8 cores), no
cross-device communication. Each core computes, for its (b, h):

  norm   = GroupNorm(x[b])                      [64, 3072]  (fp32 stats)
  q4/k4  = replicated head projections           [128, 3072] bf16
           (4 copies of q/k stacked in 32-partition strips so QK^T can
            use tile_position row-packing with K=16)
  S^T    = K^T Q  computed j-on-partitions       (no transposes needed)
  E      = exp(S^T)                              bf16 (no max subtraction:
                                                  |S| < ~40, fp32-safe)
  out    = [Mvo @ norm ; 1]^T-weighted sum of E  [65, 3072] fp32
           rows 0:64 = w_out-projected attention numerator,
           row 64    = softmax denominator (ones column trick)

Host then computes x + b_out + sum_h(num/den) and reshapes.
"""

import sys
from contextlib import ExitStack

import numpy as np
import ml_dtypes

sys.path.insert(0, "/opt/trn_rl_repo")

import concourse.bacc as bacc  # noqa: E402
import concourse.bass as bass  # noqa: E402
import concourse.tile as tile  # noqa: E402
from concourse import mybir  # noqa: E402
from concourse.bass_utils import run_bass_kernel_spmd  # noqa: E402

B, C, D_, H_, W_ = 2, 64, 12, 16, 16
N = D_ * H_ * W_  # 3072
NH, DH, NG = 4, 16, 4  # heads, head_dim, groups
EPS = 1e-5
F32 = mybir.dt.float32
BF16 = mybir.dt.bfloat16

NCHUNK = 512
NCH = N // NCHUNK  # 6 i-chunks
JBLK = 128
NJB = N // JBLK  # 24 j-blocks
PACK = 3  # j-blocks per PSUM pack (3 banks; x2 buffers + 2 PV banks = 8)
NPACKS = NJB // PACK  # 8


def build_program():
    nc = bacc.Bacc("TRN2", target_bir_lowering=False)

    xb_d = nc.dram_tensor("xb", [C, N], F32, kind="ExternalInput")
    wq4_d = nc.dram_tensor("wq4", [C, 128], BF16, kind="ExternalInput")
    wk4_d = nc.dram_tensor("wk4", [C, 128], BF16, kind="ExternalInput")
    mvoT_d = nc.dram_tensor("mvoT", [C, C], BF16, kind="ExternalInput")
    gnw_d = nc.dram_tensor("gnw", [C, 1], F32, kind="ExternalInput")
    gnb_d = nc.dram_tensor("gnb", [C, 1], F32, kind="ExternalInput")
    gmask_d = nc.dram_tensor("gmask", [C, NG], F32, kind="ExternalInput")
    gmaskT_d = nc.dram_tensor("gmaskT", [NG, C], F32, kind="ExternalInput")
    out_d = nc.dram_tensor("out", [C + 1, N], F32, kind="ExternalOutput")

    with tile.TileContext(nc) as tc, ExitStack() as ctx:
        consts = ctx.enter_context(tc.tile_pool(name="consts", bufs=1))
        work = ctx.enter_context(tc.tile_pool(name="work", bufs=1))
        small = ctx.enter_context(tc.tile_pool(name="small", bufs=2))
        epool = ctx.enter_context(tc.tile_pool(name="epool", bufs=3))
        opool = ctx.enter_context(tc.tile_pool(name="opool", bufs=2))
        psum = ctx.enter_context(tc.tile_pool(name="psum", bufs=2, space="PSUM"))

        # ---- PE warmup ----
        # The HAM clock gate keeps PE at 1.2 GHz until ~3.4us of sustained
        # matmul activity. Fill the (otherwise idle) DMA/stats preamble with
        # dummy matmuls so the real loop starts at 2.4 GHz.
        wz_l = consts.tile([128, 128], BF16, tag="wz_l")
        nc.vector.memset(wz_l, 0.0)
        wz_r = consts.tile([128, NCHUNK], BF16, tag="wz_r")
        nc.vector.memset(wz_r, 0.0)
        wps = psum.tile([128, NCHUNK], F32, tag="sp")
        for _ in range(16):
            nc.tensor.matmul(out=wps, lhsT=wz_l, rhs=wz_r, start=True, stop=True)

        # ---- load inputs ----
        xs = work.tile([C, N], F32, tag="xs")
        wq4 = consts.tile([C, 128], BF16, tag="wq4")
        nc.sync.dma_start(out=wq4, in_=wq4_d[:, :])
        wk4 = consts.tile([C, 128], BF16, tag="wk4")
        nc.sync.dma_start(out=wk4, in_=wk4_d[:, :])
        mvoT = consts.tile([C, C], BF16, tag="mvoT")
        nc.sync.dma_start(out=mvoT, in_=mvoT_d[:, :])
        gnw = consts.tile([C, 1], F32, tag="gnw")
        nc.sync.dma_start(out=gnw, in_=gnw_d[:, :])
        gnb = consts.tile([C, 1], F32, tag="gnb")
        nc.sync.dma_start(out=gnb, in_=gnb_d[:, :])
        gmask = consts.tile([C, NG], F32, tag="gmask")
        nc.sync.dma_start(out=gmask, in_=gmask_d[:, :])
        gmaskT = consts.tile([NG, C], F32, tag="gmaskT")
        nc.sync.dma_start(out=gmaskT, in_=gmaskT_d[:, :])
        eps_t = consts.tile([C, 1], F32, tag="eps_t")
        nc.vector.memset(eps_t, EPS)

        # ---- GroupNorm statistics ----
        # per-channel mean/var over the 3072 free elements; x is DMA'd in
        # 512-column chunks so bn_stats overlaps the transfer
        xs_g = xs.rearrange("p (n f) -> p n f", f=512)
        stats = small.tile([C, N // 512, 6], F32, tag="stats")
        for s in range(N // 512):
            nc.sync.dma_start(out=xs_g[:, s, :], in_=xb_d[:, s * 512 : (s + 1) * 512])
            nc.vector.bn_stats(out=stats[:, s, :], in_=xs_g[:, s, :])
        mv = small.tile([C, 2], F32, tag="mv")
        nc.vector.bn_aggr(out=mv, in_=stats)
        # stat2: col0 = mean_c, col1 = mean_c^2 + var_c  (= E[x_c^2])
        stat2 = small.tile([C, 2], F32, tag="stat2")
        nc.vector.tensor_copy(out=stat2[:, 0:1], in_=mv[:, 0:1])
        nc.vector.tensor_mul(out=stat2[:, 1:2], in0=mv[:, 0:1], in1=mv[:, 0:1])
        nc.vector.tensor_add(out=stat2[:, 1:2], in0=stat2[:, 1:2], in1=mv[:, 1:2])
        # cross-partition group reduction via mask matmuls
        gm_ps = psum.tile([NG, 2], F32, tag="pv")
        nc.tensor.matmul(out=gm_ps, lhsT=gmask, rhs=stat2, start=True, stop=True)
        gm_sb = small.tile([NG, 2], F32, tag="gm_sb")
        nc.vector.tensor_copy(out=gm_sb, in_=gm_ps)
        gb_ps = psum.tile([C, 2], F32, tag="pv")
        nc.tensor.matmul(out=gb_ps, lhsT=gmaskT, rhs=gm_sb, start=True, stop=True)
        gb = small.tile([C, 2], F32, tag="gb")
        nc.vector.tensor_copy(out=gb, in_=gb_ps)
        # var_g = E[x^2] - mean^2 ; rstd = exp(-0.5*ln(var+eps))
        var = small.tile([C, 1], F32, tag="var")
        nc.vector.tensor_mul(out=var, in0=gb[:, 0:1], in1=gb[:, 0:1])
        nc.vector.tensor_sub(out=var, in0=gb[:, 1:2], in1=var)
        logv = small.tile([C, 1], F32, tag="logv")
        nc.scalar.activation(
            out=logv, in_=var, func=mybir.ActivationFunctionType.Ln, bias=eps_t
        )
        rstd = small.tile([C, 1], F32, tag="rstd")
        nc.scalar.activation(
            out=rstd, in_=logv, func=mybir.ActivationFunctionType.Exp, scale=-0.5
        )
        scale = small.tile([C, 1], F32, tag="scale")
        nc.vector.tensor_mul(out=scale, in0=rstd, in1=gnw)
        nbias = small.tile([C, 1], F32, tag="nbias")
        nc.vector.tensor_mul(out=nbias, in0=gb[:, 0:1], in1=scale)
        nc.vector.tensor_sub(out=nbias, in0=gnb, in1=nbias)
        # norm = x*scale + nbias, cast to bf16
        norm = work.tile([C, N], BF16, tag="norm")
        nc.vector.tensor_scalar(
            out=norm,
            in0=xs,
            scalar1=scale,
            scalar2=nbias,
            op0=mybir.AluOpType.mult,
            op1=mybir.AluOpType.add,
        )

        # ---- Q/K (4x replicated along partition strips) and G ----
        q4 = work.tile([128, N], BF16, tag="q4")
        k4 = work.tile([128, N], BF16, tag="k4")
        for dst, wmat in ((q4, wq4), (k4, wk4)):
            for half in range(2):
                ps = psum.tile([128, PACK * NCHUNK], F32, tag="sp")
                for cc in range(3):
                    ic = half * 3 + cc
                    nc.tensor.matmul(
                        out=ps[:, cc * NCHUNK : (cc + 1) * NCHUNK],
                        lhsT=wmat,
                        rhs=norm[:, ic * NCHUNK : (ic + 1) * NCHUNK],
                        start=True,
                        stop=True,
                    )
                nc.any.tensor_copy(
                    out=dst[:, half * 3 * NCHUNK : (half + 1) * 3 * NCHUNK], in_=ps
                )
        # G[j, 0:64] = (w_out_h @ w_v_h @ norm)^T blocks ; G[j, 64] = 1
        gsb = work.tile([128, NJB, C + 1], BF16, tag="gsb")
        gp = psum.tile([128, NJB, C], F32, tag="sp")
        for jb in range(NJB):
            nc.tensor.matmul(
                out=gp[:, jb, :],
                lhsT=norm[:, jb * JBLK : (jb + 1) * JBLK],
                rhs=mvoT,
                start=True,
                stop=True,
            )
        nc.any.tensor_copy(out=gsb[:, :, 0:C], in_=gp)
        nc.vector.memset(gsb[:, :, C : C + 1], 1.0)

        # ---- main attention loop ----
        for ic in range(NCH):
            pv = psum.tile([C + 1, NCHUNK], F32, tag="pv")
            for jg in range(NPACKS):
                sp = psum.tile([128, PACK * NCHUNK], F32, tag="sp")
                for t in range(PACK):
                    jb = jg * PACK + t
                    nc.tensor.matmul(
                        out=sp[:, t * NCHUNK : (t + 1) * NCHUNK],
                        lhsT=k4[32 * t : 32 * t + DH, jb * JBLK : (jb + 1) * JBLK],
                        rhs=q4[32 * t : 32 * t + DH, ic * NCHUNK : (ic + 1) * NCHUNK],
                        start=True,
                        stop=True,
                        tile_position=(32 * t, 0),
                    )
                ep = epool.tile([128, PACK * NCHUNK], BF16, tag="ep")
                nc.scalar.activation(
                    out=ep, in_=sp, func=mybir.ActivationFunctionType.Exp
                )
                for t in range(PACK):
                    jb = jg * PACK + t
                    nc.tensor.matmul(
                        out=pv,
                        lhsT=gsb[:, jb, :],
                        rhs=ep[:, t * NCHUNK : (t + 1) * NCHUNK],
                        start=(jb == 0),
                        stop=(jb == NJB - 1),
                    )
            ostage = opool.tile([C + 1, NCHUNK], F32, tag="ostage")
            nc.vector.tensor_copy(out=ostage, in_=pv)
            nc.sync.dma_start(
                out=out_d[:, ic * NCHUNK : (ic + 1) * NCHUNK], in_=ostage
            )

    nc.compile()
    return nc


_prog_cache = {}


def _get_program():
    if "nc" not in _prog_cache:
        _prog_cache["nc"] = build_program()
    return _prog_cache["nc"]


def _make_in_maps(x, gn_weight, gn_bias, w_qkv, w_out):
    xf = np.ascontiguousarray(x.reshape(B, C, N), np.float32)
    gnw = np.ascontiguousarray(gn_weight.reshape(C, 1), np.float32)
    gnb = np.ascontiguousarray(gn_bias.reshape(C, 1), np.float32)
    gmask = np.zeros((C, NG), np.float32)
    gmaskT = np.zeros((NG, C), np.float32)
    for c in range(C):
        gmask[c, c // DH] = 1.0 / DH
        gmaskT[c // DH, c] = 1.0
    in_maps = []
    for core in range(B * NH):
        b, h = divmod(core, NH)
        wq = w_qkv[h * DH : (h + 1) * DH, :]  # [16, 64]
        wk = w_qkv[C + h * DH : C + (h + 1) * DH, :]
        wv = w_qkv[2 * C + h * DH : 2 * C + (h + 1) * DH, :]
        wo = w_out[:, h * DH : (h + 1) * DH]  # [64, 16]
        wq4 = np.zeros((C, 128), np.float32)
        wk4 = np.zeros((C, 128), np.float32)
        for t in range(4):
            wq4[:, 32 * t : 32 * t + DH] = wq.T
            wk4[:, 32 * t : 32 * t + DH] = wk.T
        mvoT = (wo.astype(np.float64) @ wv.astype(np.float64)).T.astype(np.float32)
        in_maps.append(
            {
                "xb": xf[b].copy(),
                "wq4": wq4.astype(ml_dtypes.bfloat16),
                "wk4": wk4.astype(ml_dtypes.bfloat16),
                "mvoT": mvoT.astype(ml_dtypes.bfloat16),
                "gnw": gnw,
                "gnb": gnb,
                "gmask": gmask,
                "gmaskT": gmaskT,
            }
        )
    return in_maps


def _combine(results, x, b_out):
    xf = x.reshape(B, C, N).astype(np.float32)
    out = np.zeros((B, C, N), np.float32)
    for core in range(B * NH):
        b = core // NH
        o = np.asarray(results[core]["out"], np.float32)  # [65, N]
        out[b] += o[0:C] / o[C : C + 1]
    out += b_out.astype(np.float32)[None, :, None] + xf
    return out.reshape(B, C, D_, H_, W_).astype(np.float32)


def kernel(x, gn_weight, gn_bias, w_qkv, w_out, b_out, **_ignored):
    x = np.asarray(x, np.float32)
    w_qkv = np.asarray(w_qkv, np.float32)
    w_out = np.asarray(w_out, np.float32)
    b_out = np.asarray(b_out, np.float32)
    gn_weight = np.asarray(gn_weight, np.float32)
    gn_bias = np.asarray(gn_bias, np.float32)

    nc = _get_program()
    in_maps = _make_in_maps(x, gn_weight, gn_bias, w_qkv, w_out)
    res = run_bass_kernel_spmd(nc, in_maps, core_ids=list(range(B * NH)))
    return _combine(res.results, x, b_out)


if __name__ == "__main__":
    import reference

    inputs = {k: np.asarray(v) for k, v in reference.setup_inputs().items()}
    actual = kernel(**inputs)
    print("kernel output shape:", actual.shape, actual.dtype)
